# revision 7
# baseline (speedup 1.0000x reference)
"""GPNConv (GNN message passing) Trainium2 Bass kernel.

  agg = segment_sum(x[col], row, N)        # [N, 128]
  out = (x + agg) @ W.T + b                # [N, 512]

Sharding: destination nodes split across 8 cores (12500 each); no
cross-core communication.

Per core, the 12500 destination nodes are packed into 50 "pairs" of 250
slots by a degree-balanced greedy assignment so that every (pair, source
-bucket) group holds just under 384 edges; with equal 25000-row source
buckets (dma_gather indices are int16) this makes the chunk budget a
uniform 3 chunks of 128 edges per group (~2.5% gather padding vs 67%
for the naive node//250 split).

Each core bulk-gathers its neighbor rows x[col] from a replicated bf16
x via dma_gather, segment-sums them with one-hot matmuls on the PE
(one-hot built on-chip in bf16 via tensor_scalar is_equal in the 4x DVE
mode), adds the bf16 residual from a host-permuted-transposed x-shard,
applies the 128->512 linear (bf16 weights) with the bias folded in as a
K=1 matmul, evacuates PSUM through the scalar (ACT) engine as bf16, and
writes a bf16 output shard that the host unpermutes and casts to f32.

The chunk schedule is data-dependent but made uniform across cores by
taking the max over cores, so one SPMD program serves all 8 cores.
"""

import numpy as np

import concourse.bass as bass
import concourse.mybir as mybir
import concourse.tile as tile
from concourse import bacc
from concourse import bass_utils

P = 128
N_NODES = 100000
D_IN = 128
D_OUT = 512
N_CORES = 8
NODES_PER_CORE = N_NODES // N_CORES             # 12500
DPAIR = 250                                      # dest window (one-hot width)
PAIRS_PER_CORE = NODES_PER_CORE // DPAIR        # 50 (exact)
PAD_NODES = PAIRS_PER_CORE * DPAIR              # 12500
WAVE_PAIRS = 5                                   # pairs per gather wave
N_WAVES = PAIRS_PER_CORE // WAVE_PAIRS          # 10
BUCKET = 25000                                   # equalized source windows
N_BUCKETS = 4
PAD_SLOT = 384.0                                 # one-hot slot matching nothing

_F32 = mybir.dt.float32
_BF16 = mybir.dt.bfloat16
_I16 = mybir.dt.int16


def _balance_pairs(deg):
    """Greedy 4-D balanced assignment of nodes to pairs.

    deg: [NODES_PER_CORE, N_BUCKETS] per-node edge counts by source bucket.
    Returns perm[node] = pair*DPAIR + slot.
    """
    n = deg.shape[0]
    order = np.argsort(-deg.sum(axis=1), kind="stable")
    load = np.zeros((PAIRS_PER_CORE, N_BUCKETS), dtype=np.int64)
    slots = np.zeros(PAIRS_PER_CORE, dtype=np.int64)
    perm = np.empty(n, dtype=np.int64)
    for node in order:
        d = deg[node]
        cost = (load + d).max(axis=1).astype(np.float64)
        cost[slots >= DPAIR] = np.inf
        p = int(np.argmin(cost))
        perm[node] = p * DPAIR + slots[p]
        load[p] += d
        slots[p] += 1
    return perm


def _host_prep(edge_index):
    """Group edges by (core, pair, bucket); build uniform chunk schedule,
    int16 gather-index array and f32 dest-slot array per core."""
    row = np.asarray(edge_index[0], dtype=np.int64)
    col = np.asarray(edge_index[1], dtype=np.int64)

    core = row // NODES_PER_CORE
    local = row % NODES_PER_CORE
    bucket = col // BUCKET
    brel = (col % BUCKET).astype(np.int16)

    # per-core degree-balanced node -> (pair, slot) permutation
    perms = np.empty((N_CORES, NODES_PER_CORE), dtype=np.int64)
    nb = np.bincount(
        (core * NODES_PER_CORE + local) * N_BUCKETS + bucket,
        minlength=N_CORES * NODES_PER_CORE * N_BUCKETS,
    ).reshape(N_CORES, NODES_PER_CORE, N_BUCKETS)
    for c in range(N_CORES):
        perms[c] = _balance_pairs(nb[c])

    pp = perms[core, local]                       # permuted local position
    pair = pp // DPAIR
    pslot = pp % DPAIR

    # group key and counts
    key = (core * PAIRS_PER_CORE + pair) * N_BUCKETS + bucket
    ngroups = N_CORES * PAIRS_PER_CORE * N_BUCKETS
    counts = np.bincount(key, minlength=ngroups).reshape(
        N_CORES, PAIRS_PER_CORE, N_BUCKETS
    )
    budget = -(-counts.max(axis=0) // P)          # [PAIRS, NB] ceil
    budget[:, 0] = np.maximum(budget[:, 0], 1)    # >=1 chunk per pair

    order = np.argsort(key, kind="stable")
    brel_s = brel[order]
    pslot_s = pslot[order]
    key_s = key[order]
    starts = np.searchsorted(key_s, np.arange(ngroups + 1))

    waves = [
        list(range(w * WAVE_PAIRS, min((w + 1) * WAVE_PAIRS, PAIRS_PER_CORE)))
        for w in range(N_WAVES)
    ]
    # schedule structures (identical across cores)
    gathers = []      # per (w,b): dict(nch, qoff, coff, bucket, wave)
    pair_chunks = {}  # pair -> list of (gather_idx, local_chunk, ci)
    ci = 0            # global chunk counter
    qcols = 0         # idx tile columns consumed (num_idxs/16 each)
    for w, wp in enumerate(waves):
        for b in range(N_BUCKETS):
            nch = int(sum(budget[p][b] for p in wp))
            if nch == 0:
                continue
            g = dict(w=w, b=b, nch=nch, qoff=qcols, coff=ci)
            gathers.append(g)
            lc = 0
            for p in wp:
                for j in range(int(budget[p][b])):
                    pair_chunks.setdefault(p, []).append((len(gathers) - 1, lc, ci))
                    lc += 1
                    ci += 1
            qcols += nch * 8  # (nch*128 idxs)/16
    TC = ci

    idx_all = np.zeros((N_CORES, 16, qcols), dtype=np.int16)
    dfl_all = np.full((N_CORES, P, TC), PAD_SLOT, dtype=np.float32)
    for c in range(N_CORES):
        for g in gathers:
            w, b = g["w"], g["b"]
            lc = 0
            for p in waves[w]:
                gk = (c * PAIRS_PER_CORE + p) * N_BUCKETS + b
                b0, b1 = starts[gk], starts[gk + 1]
                n = b1 - b0
                kb = int(budget[p][b])
                assert n <= kb * P
                if n:
                    # edge j -> gather idx position i = (lc + j//128)*128 + j%128
                    i = (lc + np.arange(n) // P) * P + np.arange(n) % P
                    idx_all[c, i % 16, g["qoff"] + i // 16] = brel_s[b0:b1]
                    dfl_all[c, np.arange(n) % P, g["coff"] + lc + np.arange(n) // P] = (
                        pslot_s[b0:b1]
                    )
                lc += kb
    # replicate idx rows to 128 partitions (8 Q7 cores x 16-partition stripes)
    idx_all = np.tile(idx_all, (1, 8, 1))
    return idx_all, dfl_all, gathers, pair_chunks, waves, TC, qcols, perms


def _build_program(gathers, pair_chunks, waves, TC, qcols):
    nc = bacc.Bacc(
        "TRN2",
        target_bir_lowering=False,
        debug=False,
        enable_asserts=False,
        num_devices=N_CORES,
    )
    x_d = nc.dram_tensor("x", [N_NODES, D_IN], _BF16, kind="ExternalInput").ap()
    idx_d = nc.dram_tensor("idx", [P, qcols], _I16, kind="ExternalInput").ap()
    dfl_d = nc.dram_tensor("dfl", [P, TC], _F32, kind="ExternalInput").ap()
    xt_d = nc.dram_tensor("xt", [P, PAD_NODES], _BF16, kind="ExternalInput").ap()
    wt_d = nc.dram_tensor("wt", [P, D_OUT], _BF16, kind="ExternalInput").ap()
    bias_d = nc.dram_tensor("bias", [1, D_OUT], _BF16, kind="ExternalInput").ap()
    ones_d = nc.dram_tensor("ones", [1, P], _BF16, kind="ExternalInput").ap()
    iota_d = nc.dram_tensor("iota", [P, DPAIR], _BF16, kind="ExternalInput").ap()
    out_d = nc.dram_tensor("out", [PAD_NODES, D_OUT], _BF16, kind="ExternalOutput").ap()

    # slot-range split for the 128->512 linear: M tiles of <=128 slots
    MT = [(0, P), (P, DPAIR)]  # [(m0, m1)]

    with tile.TileContext(nc) as tc:
        with (
            tc.tile_pool(name="const", bufs=1) as cpool,
            tc.tile_pool(name="gather", bufs=2) as gpool,
            tc.tile_pool(name="xtw", bufs=2) as xtpool,
            tc.tile_pool(name="oh", bufs=10) as ohpool,
            tc.tile_pool(name="ht", bufs=4) as htpool,
            tc.tile_pool(name="ot", bufs=6) as otpool,
            tc.tile_pool(name="psA", bufs=4, space="PSUM") as psA,
            tc.tile_pool(name="psB", bufs=4, space="PSUM") as psB,
        ):
            wt_t = cpool.tile([P, D_OUT], _BF16)
            nc.sync.dma_start(out=wt_t[:], in_=wt_d)
            bias_t = cpool.tile([1, D_OUT], _BF16)
            nc.sync.dma_start(out=bias_t[:], in_=bias_d)
            ones_t = cpool.tile([1, P], _BF16)
            nc.sync.dma_start(out=ones_t[:], in_=ones_d)
            iota_t = cpool.tile([P, DPAIR], _BF16)
            nc.sync.dma_start(out=iota_t[:], in_=iota_d)
            idx_t = cpool.tile([P, qcols], _I16)
            nc.sync.dma_start(out=idx_t[:], in_=idx_d)
            dfl_t = cpool.tile([P, TC], _F32)
            nc.sync.dma_start(out=dfl_t[:], in_=dfl_d)

            for w, wp in enumerate(waves):
                gts = {}
                for g in gathers:
                    if g["w"] != w:
                        continue
                    b = g["b"]
                    nch = g["nch"]
                    b0 = b * BUCKET
                    b1 = min(b0 + BUCKET, N_NODES)
                    gt = gpool.tile([P, nch * P], _BF16, tag=f"g{b}")
                    nc.gpsimd.dma_gather(
                        gt[:].rearrange("p (c e) -> p c e", e=P),
                        x_d[b0:b1, :],
                        idx_t[:, g["qoff"] : g["qoff"] + nch * 8],
                        nch * P,
                        nch * P,
                        P,
                        single_packet=False,
                    )
                    gts[b] = gt
                xtw = xtpool.tile([P, len(wp) * DPAIR], _BF16)
                nc.sync.dma_start(
                    out=xtw[:],
                    in_=xt_d[:, wp[0] * DPAIR : (wp[-1] + 1) * DPAIR],
                )
                for pi, p in enumerate(wp):
                    chunks = pair_chunks[p]
                    psT = psA.tile([P, DPAIR], _F32)
                    for k, (gi, lc, ci) in enumerate(chunks):
                        oh = ohpool.tile([P, DPAIR], _BF16)
                        nc.vector.tensor_scalar(
                            out=oh[:],
                            in0=iota_t[:],
                            scalar1=dfl_t[:, ci : ci + 1],
                            scalar2=None,
                            op0=mybir.AluOpType.is_equal,
                        )
                        nc.tensor.matmul(
                            out=psT[:],
                            lhsT=gts[gathers[gi]["b"]][:, lc * P : (lc + 1) * P],
                            rhs=oh[:],
                            start=(k == 0),
                            stop=(k == len(chunks) - 1),
                        )
                    ht = htpool.tile([P, DPAIR], _BF16)
                    nc.vector.tensor_add(
                        out=ht[:],
                        in0=psT[:],
                        in1=xtw[:, pi * DPAIR : (pi + 1) * DPAIR],
                    )
                    for m0, m1 in MT:
                        m = m1 - m0
                        psO = psB.tile([P, D_OUT], _F32)
                        nc.tensor.matmul(
                            out=psO[:m, :], lhsT=ones_t[:, :m], rhs=bias_t[:],
                            start=True, stop=False,
                        )
                        nc.tensor.matmul(
                            out=psO[:m, :], lhsT=ht[:, m0:m1], rhs=wt_t[:],
                            start=False, stop=True,
                        )
                        ot = otpool.tile([P, D_OUT], _BF16)
                        nc.scalar.copy(out=ot[:m, :], in_=psO[:m, :])
                        r0 = p * DPAIR + m0
                        nc.sync.dma_start(out=out_d[r0 : r0 + m, :], in_=ot[:m, :])
    nc.compile()
    return nc


def prepare(inputs):
    """Build (nc, in_maps, postprocess) for this problem instance."""
    import ml_dtypes
    x = np.ascontiguousarray(np.asarray(inputs["x"], dtype=np.float32))
    xb = np.ascontiguousarray(x.astype(ml_dtypes.bfloat16))
    W = np.asarray(inputs["W"], dtype=np.float32)
    b = np.asarray(inputs["b"], dtype=np.float32)

    idx_all, dfl_all, gathers, pair_chunks, waves, TC, qcols, perms = _host_prep(
        inputs["edge_index"]
    )

    WT = np.ascontiguousarray(W.T.astype(ml_dtypes.bfloat16))
    bias_row = np.ascontiguousarray(b[None, :].astype(ml_dtypes.bfloat16))
    ones_row = np.ones((1, P), dtype=ml_dtypes.bfloat16)
    iota = np.ascontiguousarray(
        np.broadcast_to(
            np.arange(DPAIR, dtype=np.float32)[None, :], (P, DPAIR)
        ).astype(ml_dtypes.bfloat16)
    )

    in_maps = []
    for c in range(N_CORES):
        xc = x[c * NODES_PER_CORE : (c + 1) * NODES_PER_CORE]
        xt = np.zeros((P, PAD_NODES), dtype=np.float32)
        xt[:, perms[c]] = xc.T
        in_maps.append(
            {
                "x": xb,
                "idx": np.ascontiguousarray(idx_all[c]),
                "dfl": np.ascontiguousarray(dfl_all[c]),
                "xt": np.ascontiguousarray(xt.astype(ml_dtypes.bfloat16)),
                "wt": WT,
                "bias": bias_row,
                "ones": ones_row,
                "iota": iota,
            }
        )

    nc = _build_program(gathers, pair_chunks, waves, TC, qcols)

    def postprocess(results):
        out = np.concatenate(
            [
                results[c]["out"].astype(np.float32)[perms[c]]
                for c in range(N_CORES)
            ],
            axis=0,
        )
        return out

    return nc, in_maps, postprocess


def _run(inputs, trace=False):
    nc, in_maps, postprocess = prepare(inputs)
    res = bass_utils.run_bass_kernel_spmd(
        nc, in_maps, core_ids=list(range(N_CORES)), trace=trace
    )
    return postprocess(res.results), res


def kernel(**inputs):
    out, _ = _run(inputs, trace=False)
    return out
